# revision 1
# baseline (speedup 1.0000x reference)
"""Trainium2 Bass kernel for CrossFeature: out[b, p(i,j)] = x[b,i]*x[b,j]*dot(v[i],v[j]).

Full shapes: x [8192, 300] f32, v [300, 4] f32 -> out [8192, 44850] f32
(P = 300*299/2 upper-triangular pairs, row-major order).

Strategy (data-parallel over 8 NeuronCores, batch-sharded):
  - host: w[p] = (v @ v.T)[i(p), j(p)]  (tiny), shard x by batch.
  - per core (1024 rows): SBUF holds x as [128 part, 8 bh, 300] (row = bh*128+bl).
    Loop over output column chunks [c0, c1):
      * PE broadcasts the w chunk into PSUM via ones[1,128]^T @ w[1,chunk]
        (idle engine; avoids 22MB of broadcast DMA and keeps DVE's SBUF rd1
        port free for GPSIMD).
      * pass 1 (per pair-segment (i, bh)): t = x[:, bh, i+1: ] * x[:, bh, i]
        (per-partition scalar). Large segments (small i) -> ScalarE
        activation(Copy, scale); the rest -> GPSIMD tensor_scalar.
      * pass 2 (per bh): t *= w_psum (DVE tensor_tensor, in-place, PSUM operand).
      * one big HWDGE DMA of [128, 8, chunk] to the output shard.
  - No cross-core communication.
"""

import numpy as np

import concourse.bacc as bacc
import concourse.bass as bass
import concourse.mybir as mybir
from concourse.tile import TileContext
from concourse.bass_utils import run_bass_kernel_spmd

N_CORES = 8
B_FULL = 8192
F_FULL = 300

# tuning knobs
CHUNK = 1024          # output columns per tile/DMA
ACT_I_END = 120       # segments with i < this -> ScalarE, per (i, bh)
DVE_I_END = 215       # ACT_I_END <= i < this -> DVE per-i broadcast TT
                      # i >= DVE_I_END -> GPSIMD per-i broadcast TT


def bcast_last(ap, n):
    """[..., 1] AP -> [..., n] with stride-0 last dim (free-dim broadcast)."""
    a = [list(d) for d in ap.ap]
    assert a[-1][1] == 1, a
    return bass.AP(ap.tensor, ap.offset, a[:-1] + [[0, n]])


def chunk_segments(f, c0, c1):
    """Pair-segments of the triu(f, k=1) row-major layout intersected with
    column window [c0, c1). Yields (i, ps, pe, j0): output cols [ps, pe) hold
    x[:, i] * x[:, j0 : j0 + (pe-ps)]."""
    s = 0
    for i in range(f - 1):
        ln = f - 1 - i
        s0, s1 = s, s + ln
        if s0 >= c1:
            break
        if s1 > c0:
            ps, pe = max(s0, c0), min(s1, c1)
            yield i, ps, pe, i + 1 + (ps - s0)
        s = s1


def build_program(bh=8, f=F_FULL, chunk=CHUNK, act_i_end=ACT_I_END,
                  dve_i_end=DVE_I_END, n_cores=N_CORES):
    """Build + compile the per-core Bass program. Shard shape: [bh*128, f]."""
    p_pairs = f * (f - 1) // 2
    rows = bh * 128
    f32 = mybir.dt.float32

    nc = bacc.Bacc("TRN2", target_bir_lowering=False, debug=False,
                   num_devices=n_cores)
    x_d = nc.dram_tensor("x", [rows, f], f32, kind="ExternalInput")
    w_d = nc.dram_tensor("w", [1, p_pairs], f32, kind="ExternalInput")
    o_d = nc.dram_tensor("out", [rows, p_pairs], f32, kind="ExternalOutput")

    with TileContext(nc) as tc:
        with (
            tc.tile_pool(name="xp", bufs=1) as xp,
            tc.tile_pool(name="wp", bufs=3) as wp,
            tc.tile_pool(name="op", bufs=3) as op,
            tc.tile_pool(name="pp", bufs=2, space=bass.MemorySpace.PSUM) as pp,
        ):
            x_sb = xp.tile([128, bh, f], f32)
            nc.sync.dma_start(
                out=x_sb[:], in_=x_d.rearrange("(bh bl) f -> bl bh f", bl=128)
            )
            ones = xp.tile([1, 128], f32)
            nc.vector.memset(ones[:], 1.0)

            out_r = o_d.rearrange("(bh bl) p -> bl bh p", bl=128)

            for c0 in range(0, p_pairs, chunk):
                c1 = min(c0 + chunk, p_pairs)
                cw = c1 - c0

                w_sb = wp.tile([1, chunk], f32, tag="w")
                nc.sync.dma_start(out=w_sb[:, :cw], in_=w_d[:, c0:c1])
                w_ps = pp.tile([128, chunk], f32, tag="wps")
                for n0 in range(0, cw, 512):
                    n1 = min(n0 + 512, cw)
                    nc.tensor.matmul(
                        w_ps[:, n0:n1], ones[:], w_sb[:, n0:n1],
                        start=True, stop=True,
                    )

                ob = op.tile([128, bh, chunk], f32, tag="ob")
                for i, ps, pe, j0 in chunk_segments(f, c0, c1):
                    ln = pe - ps
                    if i < act_i_end:
                        for b in range(bh):
                            nc.scalar.activation(
                                ob[:, b, ps - c0:pe - c0],
                                x_sb[:, b, j0:j0 + ln],
                                mybir.ActivationFunctionType.Copy,
                                scale=x_sb[:, b, i:i + 1],
                            )
                    else:
                        eng = nc.vector if i < dve_i_end else nc.gpsimd
                        eng.tensor_mul(
                            out=ob[:, :, ps - c0:pe - c0],
                            in0=x_sb[:, :, j0:j0 + ln],
                            in1=bcast_last(x_sb[:, :, i:i + 1], ln),
                        )
                for b in range(bh):
                    nc.vector.tensor_mul(
                        out=ob[:, b, :cw], in0=ob[:, b, :cw], in1=w_ps[:, :cw]
                    )
                nc.sync.dma_start(out=out_r[:, :, c0:c1], in_=ob[:, :, :cw])

    nc.compile()
    return nc


def pair_weights(v):
    """w[p] = dot(v[i(p)], v[j(p)]) in row-major triu order, as [1, P] f32."""
    g = v.astype(np.float64) @ v.astype(np.float64).T
    ii, jj = np.triu_indices(v.shape[0], k=1)
    return np.ascontiguousarray(g[ii, jj][None, :].astype(np.float32))


_prog_cache = {}


def _get_program():
    key = (N_CORES, F_FULL, CHUNK, ACT_I_END, DVE_I_END)
    if key not in _prog_cache:
        _prog_cache[key] = build_program()
    return _prog_cache[key]


def run(x, v, trace=False, trace_kwargs=None):
    """Run on all 8 cores; returns (out [8192, P] f32, BassKernelResults)."""
    assert x.shape == (B_FULL, F_FULL), x.shape
    nc = _get_program()
    w = pair_weights(np.asarray(v))
    xs = np.ascontiguousarray(np.asarray(x, dtype=np.float32))
    b_loc = B_FULL // N_CORES
    in_maps = [
        {"x": np.ascontiguousarray(xs[c * b_loc:(c + 1) * b_loc]), "w": w}
        for c in range(N_CORES)
    ]
    res = run_bass_kernel_spmd(
        nc, in_maps, list(range(N_CORES)), trace=trace,
        **(trace_kwargs or {}),
    )
    out = np.concatenate([res.results[c]["out"] for c in range(N_CORES)], axis=0)
    return out, res


def kernel(x, v):
    out, _ = run(x, v)
    return out

